# revision 16
# baseline (speedup 1.0000x reference)
"""Trainium2 Bass kernel: batched multi-head self-attention (B=16, N=1024, D=768, H=12).

Strategy
--------
Data-parallel over the batch: 16 batches / 8 NeuronCores = 2 batches per core.
Each core runs an identical (SPMD) Bass program over its shard.

Per-core math, all matmuls in bf16 with fp32 PSUM accumulation:
  * Host pre-transposes x to xT [D, T] (T = 2048 local tokens) and casts
    x / qkv_w / proj_w to bf16.  qkv_w's Q|K columns are permuted on host
    into (K0,Q0,K1,Q1,...,K5,Q5) slot order so the first DMA chunks carry
    exactly what the first attention head-pair needs.
      - Q^T,K^T [c, tok] = matmul(lhsT=wqkv[:, slot], rhs=xT)
      - V [tok, c]       = matmul(lhsT=xT[:, tok-tile], rhs=wqkv_v)
      - S^T [k, q]       = matmul(lhsT=K^T_h [hd, k-tile], rhs=Q^T_h [hd, q])
        head pairs 2i/2i+1 live at partition bases 0/64 -> row-tiles
        (0,0)/(64,0) of the 64x128 PE config.  Score pairs are emitted in
        BURSTS of 2 consecutive k-tiles (4 MMs back to back): entering and
        leaving the 64-row PE config costs ~100ns of pipeline drain each
        way, so amortizing 2 pairs per switch halves that tax.
      - exp on ScalarE straight out of PSUM, bf16 into SBUF
      - out^T [hd, q]    = matmul(lhsT=[V_h | ones(64)], rhs=expT [k, q]);
        psum rows 64-127 = softmax denominator, replicated for free.
      - normalize: copy denominator to SBUF (reciprocal_approx_fast's
        BITWISE_NOT seed needs raw fp32 bits; PSUM reads don't deliver
        those on HW), reciprocal, tensor_mul -> outT (bf16)
      - y^T [e, tok]     = matmul(lhsT=proj_w[:, e-block], rhs=out^T);
        ScalarE ACT (Identity + per-partition bias) drains PSUM -> bf16
        SBUF, so the projection tail never queues behind the DVE.  The
        host transposes y^T back to [tok, e] when unsharding.

Scheduling: the Tile framework scheduler is dependency-driven with a
priority heap per engine, so ordering is controlled via priority BANDS:
the attention spine (S^T bursts, exp, A@V, normalize) gets the lowest
priority numbers and preempts whenever its dependencies are satisfied;
QKV / V / proj matmul units sit in higher bands and automatically
backfill every PE bubble.  Fill bands are deadline-ordered (late-needed
units get later bands) so the final spine doesn't run dry of backfill.
DMA chunks are need-ordered at fine grain (x in qt-halves, weights in
slot-pair columns) so the PE starts ~1us in and stays busy, warming the
HAM clock gate early.

kernel() takes full unsharded inputs, shards on host, runs all 8 cores via
run_bass_kernel_spmd, and re-assembles the full output.
"""

import numpy as np
import ml_dtypes

import concourse.bass as bass
import concourse.mybir as mybir
import concourse.tile as tile
from concourse import bacc
from concourse.bass_utils import run_bass_kernel_spmd

BF16 = mybir.dt.bfloat16
F32 = mybir.dt.float32

N_CORES = 8
B, SEQ, D = 16, 1024, 768
H, HD = 12, 64
BPC = B // N_CORES            # batches per core
T = BPC * SEQ                 # tokens per core
P = 128
KT = D // P                   # 6 contraction sub-tiles of 128
NQ = 512                      # moving free-dim per matmul (1 psum bank of fp32)
QT = SEQ // NQ                # 2 query tiles per batch
KTT = SEQ // P                # 8 key-token tiles per batch
NV = 384                      # V-projection output tile (2 per 768)
SCALE = HD ** -0.5


def _emit(tc, xT_d, wqkv_d, wproj_d, bias_d, yT_d):
    nc = tc.nc
    from contextlib import ExitStack

    def band(n):
        tc.cur_priority = n

    with ExitStack() as ctx:
        consts = ctx.enter_context(tc.tile_pool(name="consts", bufs=1))
        xt_pool = ctx.enter_context(tc.tile_pool(name="xt", bufs=2))
        qk_pool = ctx.enter_context(tc.tile_pool(name="qkT", bufs=2))
        v_pool = ctx.enter_context(tc.tile_pool(name="v", bufs=2))
        ot_pool = ctx.enter_context(tc.tile_pool(name="ot", bufs=2))
        e_pool = ctx.enter_context(tc.tile_pool(name="e", bufs=5))
        dn_pool = ctx.enter_context(tc.tile_pool(name="dn", bufs=2))
        rb_pool = ctx.enter_context(tc.tile_pool(name="rb", bufs=2))
        y_pool = ctx.enter_context(tc.tile_pool(name="y", bufs=8))
        mm_ps = ctx.enter_context(tc.tile_pool(name="mmps", bufs=2, space="PSUM"))
        st_ps = ctx.enter_context(tc.tile_pool(name="stps", bufs=2, space="PSUM"))
        av_ps = ctx.enter_context(tc.tile_pool(name="avps", bufs=2, space="PSUM"))

        # ---------------- DMA (need-ordered chunks), band 0 ----------------
        band(0)
        xT_full = xT_d[:].rearrange("(po pi) t -> pi po t", pi=P)   # [128, 6, T]
        wqkv_full = wqkv_d[:].rearrange("(po pi) c -> pi po c", pi=P)

        wqkv_sb = consts.tile([P, KT, 3 * D], BF16)
        xT_sb = {}
        for b in range(BPC):
            xT_sb[b] = xt_pool.tile([P, KT, SEQ], BF16, tag="xt", name=f"xT{b}")

        # Coarse 3D-AP chunks: each dma_start costs ~600ns of HWDGE issue
        # time on the single Sync queue, so fewer+bigger wins; order is by
        # first consumer.
        def dma_w(c0, c1):
            nc.sync.dma_start(out=wqkv_sb[:, :, c0:c1],
                              in_=wqkv_full[:, :, c0:c1])

        def dma_x(b, t0, t1):
            nc.sync.dma_start(out=xT_sb[b][:, :, t0:t1],
                              in_=xT_full[:, :, b * SEQ + t0:b * SEQ + t1])

        dma_w(0, 256)                  # hp0's K|Q slots
        dma_x(0, 0, NQ)                # qt0 half of x(b0)
        dma_w(2 * D, 2 * D + NV)       # V cols nt=0 (heads 0..5)
        dma_x(0, NQ, SEQ)              # qt1 half (K qt1 + v token-tiles 4-7)
        dma_w(256, 768)                # slot-pairs 1,2
        dma_x(1, 0, NQ)
        dma_w(768, 2 * D)              # slot-pairs 3,4,5
        dma_w(2 * D + NV, 3 * D)       # V cols nt=1 (first needed by hp3)
        dma_x(1, NQ, SEQ)
        wproj_sb = consts.tile([P, KT, D], BF16)
        nc.sync.dma_start(
            out=wproj_sb, in_=wproj_d[:].rearrange("(po pi) c -> pi po c", pi=P)
        )
        # bias laid out per-partition for the yT ACT drain: bias_sb[p, e] =
        # bias[e*128 + p]
        bias_sb = consts.tile([P, KT], F32)
        nc.sync.dma_start(
            out=bias_sb, in_=bias_d[:].rearrange("(e p) -> p e", p=P)
        )

        # ---------------- per-batch state ----------------
        qkT_sb, v_sb, outT_sb = {}, {}, {}
        for b in range(BPC):
            qkT_sb[b] = qk_pool.tile([P, 2 * KT, SEQ], BF16, tag="qkT",
                                     name=f"qkT{b}")
            v_sb[b] = v_pool.tile([P, KTT, H, 2 * HD], BF16, tag="v",
                                  name=f"v{b}")
            nc.gpsimd.memset(v_sb[b][:, :, :, HD:2 * HD], 1.0)
            outT_sb[b] = ot_pool.tile([P, KT, SEQ], BF16, tag="ot",
                                      name=f"ot{b}")

        # ---------------- filler units ----------------
        def qk_unit(b, slot, qt):
            ps = mm_ps.tile([P, NQ], F32, tag="mm", name=f"qk{b}_{slot}_{qt}")
            for kt in range(KT):
                nc.tensor.matmul(
                    ps,
                    lhsT=wqkv_sb[:, kt, slot * P:(slot + 1) * P],
                    rhs=xT_sb[b][:, kt, qt * NQ:(qt + 1) * NQ],
                    start=(kt == 0),
                    stop=(kt == KT - 1),
                    skip_group_check=True,
                )
            nc.vector.tensor_copy(
                out=qkT_sb[b][:, slot, qt * NQ:(qt + 1) * NQ], in_=ps
            )

        def v_unit(b, tt, nt):
            ps = mm_ps.tile([P, NQ], F32, tag="mm", name=f"v{b}_{tt}_{nt}")
            for kt in range(KT):
                nc.tensor.matmul(
                    ps[:, :NV],
                    lhsT=xT_sb[b][:, kt, tt * P:(tt + 1) * P],
                    rhs=wqkv_sb[:, kt, 2 * D + nt * NV:2 * D + (nt + 1) * NV],
                    start=(kt == 0),
                    stop=(kt == KT - 1),
                    skip_group_check=True,
                )
            nc.vector.tensor_copy(
                out=v_sb[b][:, tt, nt * 6:(nt + 1) * 6, 0:HD],
                in_=ps[:, :NV].rearrange("p (h d) -> p h d", d=HD),
            )

        def proj_unit(b, eb, qt):
            # yT[e-block, tok-chunk] = sum_d wproj[d, e]^T @ outT[d, tok]
            # dt2-th MM only reads head-pair dt2's outT slice, so the per-MM
            # deps let most of the unit run before the spine fully finishes.
            t0 = qt * NQ
            ps = mm_ps.tile([P, NQ], F32, tag="mm", name=f"p{b}_{eb}_{qt}")
            for dt2 in range(KT):
                nc.tensor.matmul(
                    ps,
                    lhsT=wproj_sb[:, dt2, eb * P:(eb + 1) * P],
                    rhs=outT_sb[b][:, dt2, t0:t0 + NQ],
                    start=(dt2 == 0),
                    stop=(dt2 == KT - 1),
                    skip_group_check=True,
                )
            y_sb = y_pool.tile([P, NQ], BF16, tag="y", name=f"y{b}_{eb}_{qt}")
            if eb % 2 == 0:
                nc.scalar.activation(
                    out=y_sb,
                    in_=ps,
                    func=mybir.ActivationFunctionType.Identity,
                    bias=bias_sb[:, eb:eb + 1],
                    scale=1.0,
                )
            else:
                nc.vector.tensor_scalar_add(
                    out=y_sb, in0=ps, scalar1=bias_sb[:, eb:eb + 1]
                )
            nc.sync.dma_start(
                out=yT_d[eb * P:(eb + 1) * P,
                         b * SEQ + t0:b * SEQ + t0 + NQ],
                in_=y_sb,
            )

        # need-order for a batch's QKV/V units (spine consumption order).
        FILL_ORDER_B0 = [
            ("qk", 0, 0), ("qk", 1, 0),
            ("v", 0, 0), ("v", 1, 0),
            ("qk", 0, 1),
            ("v", 2, 0), ("v", 3, 0),
            ("qk", 2, 0), ("qk", 3, 0), ("qk", 2, 1),
            ("v", 4, 0), ("v", 5, 0),
            ("qk", 4, 0), ("qk", 5, 0), ("qk", 4, 1),
            ("v", 6, 0), ("v", 7, 0),
            ("qk", 1, 1),
            ("qk", 6, 0), ("qk", 7, 0), ("qk", 6, 1),
            ("v", 0, 1), ("v", 1, 1), ("v", 2, 1),
            ("qk", 3, 1),
            ("qk", 8, 0), ("qk", 9, 0), ("qk", 8, 1),
            ("v", 3, 1), ("v", 4, 1),
            ("qk", 10, 0), ("qk", 11, 0), ("qk", 10, 1),
            ("qk", 5, 1),
            ("v", 5, 1), ("v", 6, 1), ("v", 7, 1),
            ("qk", 7, 1), ("qk", 9, 1), ("qk", 11, 1),
        ]
        # b1: same order, but the odd-slot (Q) qt=1 units are deferred to a
        # late band so the final spine still has backfill (see below).
        B1_LATE = {("qk", 1, 1), ("qk", 3, 1), ("qk", 5, 1),
                   ("qk", 7, 1), ("qk", 9, 1), ("qk", 11, 1)}
        FILL_ORDER_B1 = [u for u in FILL_ORDER_B0 if u not in B1_LATE]

        band(100000)
        for kind, a, c in FILL_ORDER_B0:
            (qk_unit if kind == "qk" else v_unit)(0, a, c)
        band(200000)
        for kind, a, c in FILL_ORDER_B1:
            (qk_unit if kind == "qk" else v_unit)(1, a, c)
        # late-deadline fills: needed only at spine(b1,qt1) start; banded
        # above the proj fills of earlier (b,qt) so they survive until the
        # late spines instead of being gobbled by early bubbles.
        band(325000)
        for kind, a, c in sorted(B1_LATE, key=lambda u: u[1]):
            qk_unit(1, a, c)

        # ---------------- attention spine (lowest priorities) -------------
        def attn_unit(b, qt, hp, spine_band):
            avs = [
                av_ps.tile([P, NQ], F32, tag="av", name=f"av{b}_{hp}_{qt}_{i}")
                for i in range(2)
            ]
            epairs = {}
            stps = {}

            def st_pair(kt):
                # scores get a strictly lower priority number than the AVs so
                # the scheduler emits both pairs of a burst back-to-back (one
                # 64-row PE-config entry+exit per 4 MMs instead of per 2).
                band(spine_band)
                stp = st_ps.tile(
                    [P, 2, NQ], F32, tag="st", name=f"st{b}_{hp}_{qt}_{kt}"
                )
                for hi in range(2):
                    base = hi * HD
                    nc.tensor.matmul(
                        stp[:, hi, :],
                        lhsT=qkT_sb[b][
                            base:base + HD, 2 * hp, kt * P:(kt + 1) * P
                        ],
                        rhs=qkT_sb[b][
                            base:base + HD, 2 * hp + 1, qt * NQ:(qt + 1) * NQ
                        ],
                        start=True,
                        stop=True,
                        skip_group_check=True,
                    )
                stps[kt] = stp

            def exp_t(kt):
                band(spine_band)
                e_t = e_pool.tile(
                    [P, 2, NQ], BF16, tag="e", name=f"e{b}_{hp}_{qt}_{kt}"
                )
                nc.scalar.activation(
                    out=e_t,
                    in_=stps.pop(kt),
                    func=mybir.ActivationFunctionType.Exp,
                    scale=SCALE,
                )
                epairs[kt] = e_t

            def av(hi, kt):
                band(spine_band + 1)
                nc.tensor.matmul(
                    avs[hi],
                    lhsT=v_sb[b][:, kt, 2 * hp + hi, :],
                    rhs=epairs[kt][:, hi, :],
                    start=(kt == 0),
                    stop=(kt == KTT - 1),
                    skip_group_check=True,
                )

            def normalize(hi):
                band(spine_band + 1)
                base = hi * HD
                den = dn_pool.tile(
                    [HD, NQ], F32, tag="den", name=f"den{b}_{hp}_{qt}_{hi}"
                )
                nc.vector.tensor_copy(out=den, in_=avs[hi][HD:2 * HD, :])
                rb = rb_pool.tile(
                    [HD, NQ], F32, tag="rb", name=f"rb{b}_{hp}_{qt}_{hi}"
                )
                nc.vector.reciprocal_approx_fast(out=rb, in_=den)
                nc.vector.tensor_mul(
                    out=outT_sb[b][
                        base:base + HD, hp, qt * NQ:(qt + 1) * NQ
                    ],
                    in0=avs[hi][0:HD, :],
                    in1=rb,
                )

            # bursts of 2 score-pairs (4 MMs, one 64-config entry+exit);
            # AVs for the previous burst trail by one burst.
            st_pair(0)
            st_pair(1)
            exp_t(0)
            exp_t(1)
            for kt2 in range(2, KTT, 2):
                st_pair(kt2)
                st_pair(kt2 + 1)
                exp_t(kt2)
                exp_t(kt2 + 1)
                for kt in (kt2 - 2, kt2 - 1):
                    av(0, kt)
                    av(1, kt)
            for kt in (KTT - 2, KTT - 1):
                av(0, kt)
                av(1, kt)
            normalize(0)
            normalize(1)

        # Dependencies are tracked in EMISSION order (priorities only
        # reorder within the dep graph), so proj units — which read outT —
        # must be emitted after the spine units that write it.
        for b in range(BPC):
            for qt in range(QT):
                sb_band = 1000 + (2 * b + qt) * 1000
                band(sb_band)
                for hp in range(H // 2):
                    attn_unit(b, qt, hp, sb_band)
                band(300000 + (2 * b + qt) * 10000)
                for eb in range(KT):
                    proj_unit(b, eb, qt)


def _build_program():
    nc = bacc.Bacc()
    xT_d = nc.declare_dram_parameter("xT", [D, T], BF16, isOutput=False)
    wqkv_d = nc.declare_dram_parameter("wqkv", [D, 3 * D], BF16, isOutput=False)
    wproj_d = nc.declare_dram_parameter("wproj", [D, D], BF16, isOutput=False)
    bias_d = nc.declare_dram_parameter("bias", [D], F32, isOutput=False)
    yT_d = nc.declare_dram_parameter("yT", [D, T], BF16, isOutput=True)

    with tile.TileContext(nc) as tc:
        _emit(tc, xT_d, wqkv_d, wproj_d, bias_d, yT_d)
    nc.compile()
    return nc


_NC = None


def _get_nc():
    global _NC
    if _NC is None:
        _NC = _build_program()
    return _NC


def _qk_slot_perm():
    """Column permutation for the Q|K part of qkv_w: slot 2h <- K head-pair h,
    slot 2h+1 <- Q head-pair h."""
    perm = []
    for hp in range(H // 2):
        perm.extend(range(D + hp * P, D + (hp + 1) * P))      # K slot
        perm.extend(range(hp * P, (hp + 1) * P))              # Q slot
    return np.array(perm)


def _prep_in_maps(x, qkv_w, proj_w, proj_b):
    bf16 = ml_dtypes.bfloat16
    qkv_w = np.asarray(qkv_w)
    perm = _qk_slot_perm()
    wq_perm = np.concatenate([qkv_w[:, perm], qkv_w[:, 2 * D:]], axis=1)
    wq = np.ascontiguousarray(wq_perm.astype(bf16))
    wp = np.ascontiguousarray(np.asarray(proj_w).astype(bf16))
    pb = np.ascontiguousarray(np.asarray(proj_b).astype(np.float32))
    x = np.asarray(x)
    in_maps = []
    for c in range(N_CORES):
        xc = x[c * BPC:(c + 1) * BPC].reshape(T, D).astype(bf16)
        xTc = np.ascontiguousarray(xc.T)  # [D, T] bf16
        in_maps.append({"xT": xTc, "wqkv": wq, "wproj": wp, "bias": pb})
    return in_maps


def _run(x, qkv_w, proj_w, proj_b, **spmd_kwargs):
    nc = _get_nc()
    in_maps = _prep_in_maps(x, qkv_w, proj_w, proj_b)
    res = run_bass_kernel_spmd(nc, in_maps, core_ids=list(range(N_CORES)), **spmd_kwargs)
    # yT is [D, T] per core; transpose back to [T, D] on host.
    y = np.stack(
        [res.results[c]["yT"].astype(np.float32).T for c in range(N_CORES)]
    )
    return y.reshape(B, SEQ, D), res


def kernel(x, qkv_w, proj_w, proj_b):
    y, _ = _run(x, qkv_w, proj_w, proj_b)
    return y


# revision 25
# speedup vs baseline: 1.0930x; 1.0930x over previous
"""Trainium2 Bass kernel: batched multi-head self-attention (B=16, N=1024, D=768, H=12).

Strategy
--------
Data-parallel over the batch: 16 batches / 8 NeuronCores = 2 batches per core.
Each core runs an identical (SPMD) Bass program over its shard.

Per-core math, all matmuls in bf16 with fp32 PSUM accumulation:
  * Host pre-transposes x to xT [D, T] (T = 2048 local tokens) and casts
    x / qkv_w / proj_w to bf16.  qkv_w's Q|K columns are permuted on host
    into (K0,Q0,K1,Q1,...,K5,Q5) slot order so the first DMA chunks carry
    exactly what the first attention head-pair needs.
      - Q^T,K^T [c, tok] = matmul(lhsT=wqkv[:, slot], rhs=xT)
      - V [tok, c]       = matmul(lhsT=xT[:, tok-tile], rhs=wqkv_v)
      - S^T [k, q]       = matmul(lhsT=K^T_h [hd, k-tile], rhs=Q^T_h [hd, q])
        head pairs 2i/2i+1 live at partition bases 0/64 -> row-tiles
        (0,0)/(64,0) of the 64x128 PE config.  Score pairs are emitted in
        BURSTS of 2 consecutive k-tiles (4 MMs back to back): entering and
        leaving the 64-row PE config costs ~100ns of pipeline drain each
        way, so amortizing 2 pairs per switch halves that tax.
      - exp on ScalarE straight out of PSUM, bf16 into SBUF
      - out^T [hd, q]    = matmul(lhsT=[V_h | ones(64)], rhs=expT [k, q]);
        psum rows 64-127 = softmax denominator, replicated for free.
      - normalize: copy denominator to SBUF (reciprocal_approx_fast's
        BITWISE_NOT seed needs raw fp32 bits; PSUM reads don't deliver
        those on HW), reciprocal, tensor_mul -> outT (bf16)
      - y^T [e, tok]     = matmul(lhsT=proj_w[:, e-block], rhs=out^T);
        ScalarE ACT (Identity + per-partition bias) drains PSUM -> bf16
        SBUF, so the projection tail never queues behind the DVE.  The
        host transposes y^T back to [tok, e] when unsharding.

Scheduling: the Tile framework scheduler is dependency-driven with a
priority heap per engine, so ordering is controlled via priority BANDS:
the attention spine (S^T bursts, exp, A@V, normalize) gets the lowest
priority numbers and preempts whenever its dependencies are satisfied;
QKV / V / proj matmul units sit in higher bands and automatically
backfill every PE bubble.  Fill bands are deadline-ordered (late-needed
units get later bands) so the final spine doesn't run dry of backfill.
DMA chunks are need-ordered at fine grain (x in qt-halves, weights in
slot-pair columns) so the PE starts ~1us in and stays busy, warming the
HAM clock gate early.

kernel() takes full unsharded inputs, shards on host, runs all 8 cores via
run_bass_kernel_spmd, and re-assembles the full output.
"""

import numpy as np
import ml_dtypes

import concourse.bass as bass
import concourse.mybir as mybir
import concourse.tile as tile
from concourse import bacc
from concourse.bass_utils import run_bass_kernel_spmd

BF16 = mybir.dt.bfloat16
F32 = mybir.dt.float32

N_CORES = 8
B, SEQ, D = 16, 1024, 768
H, HD = 12, 64
BPC = B // N_CORES            # batches per core
T = BPC * SEQ                 # tokens per core
P = 128
KT = D // P                   # 6 contraction sub-tiles of 128
NQ = 512                      # moving free-dim per matmul (1 psum bank of fp32)
QT = SEQ // NQ                # 2 query tiles per batch
KTT = SEQ // P                # 8 key-token tiles per batch
NV = 384                      # V-projection output tile (2 per 768)
SCALE = HD ** -0.5


def _emit(tc, xT_d, wqkv_d, wproj_d, bias_d, yT_d):
    nc = tc.nc
    from contextlib import ExitStack

    def band(n):
        tc.cur_priority = n

    with ExitStack() as ctx:
        consts = ctx.enter_context(tc.tile_pool(name="consts", bufs=1))
        xt_pool = ctx.enter_context(tc.tile_pool(name="xt", bufs=2))
        qk_pool = ctx.enter_context(tc.tile_pool(name="qkT", bufs=2))
        v_pool = ctx.enter_context(tc.tile_pool(name="v", bufs=2))
        ot_pool = ctx.enter_context(tc.tile_pool(name="ot", bufs=2))
        e_pool = ctx.enter_context(tc.tile_pool(name="e", bufs=5))
        dn_pool = ctx.enter_context(tc.tile_pool(name="dn", bufs=2))
        rb_pool = ctx.enter_context(tc.tile_pool(name="rb", bufs=2))
        y_pool = ctx.enter_context(tc.tile_pool(name="y", bufs=8))
        mm_ps = ctx.enter_context(tc.tile_pool(name="mmps", bufs=2, space="PSUM"))
        st_ps = ctx.enter_context(tc.tile_pool(name="stps", bufs=2, space="PSUM"))
        av_ps = ctx.enter_context(tc.tile_pool(name="avps", bufs=2, space="PSUM"))

        # ---------------- DMA (need-ordered chunks), band 0 ----------------
        band(0)
        xT_full = xT_d[:].rearrange("(po pi) t -> pi po t", pi=P)   # [128, 6, T]
        wqkv_full = wqkv_d[:].rearrange("(po pi) c -> pi po c", pi=P)

        wqkv_sb = consts.tile([P, KT, 3 * D], BF16)
        xT_sb = {}
        for b in range(BPC):
            xT_sb[b] = xt_pool.tile([P, KT, SEQ], BF16, tag="xt", name=f"xT{b}")

        # Weights issue on the Scalar HWDGE queue, x on the Sync queue: the
        # ~600ns-per-dma_start descriptor-generation cost then runs on two
        # queues in parallel, so the first qk unit's inputs land ~2us
        # earlier.  Chunk sizes balance issue cost vs need-granularity.
        def dma_w(c0, c1):
            nc.scalar.dma_start(out=wqkv_sb[:, :, c0:c1],
                                in_=wqkv_full[:, :, c0:c1])

        def dma_x(b, k0, k1, t0, t1):
            nc.sync.dma_start(out=xT_sb[b][:, k0:k1, t0:t1],
                              in_=xT_full[:, k0:k1, b * SEQ + t0:b * SEQ + t1])

        dma_w(0, 256)                  # hp0's K|Q slots
        for k0 in range(0, KT, 2):     # qt0 half of x(b0), 2-kt chunks
            dma_x(0, k0, k0 + 2, 0, NQ)
        dma_w(2 * D, 2 * D + NV)       # V cols nt=0 (heads 0..5)
        for k0 in range(0, KT, 2):     # qt1 half (K qt1 + v token-tiles 4-7)
            dma_x(0, k0, k0 + 2, NQ, SEQ)
        dma_w(256, 768)                # slot-pairs 1,2
        dma_x(1, 0, KT, 0, NQ)
        dma_w(768, 2 * D)              # slot-pairs 3,4,5
        dma_w(2 * D + NV, 3 * D)       # V cols nt=1 (first needed by hp3)
        dma_x(1, 0, KT, NQ, SEQ)
        wproj_sb = consts.tile([P, KT, D], BF16)
        nc.scalar.dma_start(
            out=wproj_sb, in_=wproj_d[:].rearrange("(po pi) c -> pi po c", pi=P)
        )
        # bias laid out per-partition for the yT drain: bias_sb[p, e] =
        # bias[e*128 + p]
        bias_sb = consts.tile([P, KT], F32)
        nc.scalar.dma_start(
            out=bias_sb, in_=bias_d[:].rearrange("(e p) -> p e", p=P)
        )

        # HAM warm-up: the PE clock-gate defaults to half rate and needs
        # ~3.4us of sustained matmul activity to lift; a dozen garbage
        # matmuls on a zeroed scratch tile flip it before the first real
        # unit's inputs even arrive (results are never read).
        ws = consts.tile([P, NQ], BF16)
        nc.gpsimd.memset(ws, 0.0)
        for w_i in range(20):
            wps = mm_ps.tile([P, NQ], F32, tag="mm", name=f"warm{w_i}")
            nc.tensor.matmul(
                wps[:, 0:2 * P],
                lhsT=ws[:, 0:P],
                rhs=ws[:, 0:2 * P],
                start=True,
                stop=True,
                skip_group_check=True,
            )

        # ---------------- per-batch state ----------------
        qkT_sb, v_sb, outT_sb = {}, {}, {}
        for b in range(BPC):
            qkT_sb[b] = qk_pool.tile([P, 2 * KT, SEQ], BF16, tag="qkT",
                                     name=f"qkT{b}")
            v_sb[b] = v_pool.tile([P, KTT, H, 2 * HD], BF16, tag="v",
                                  name=f"v{b}")
            nc.gpsimd.memset(v_sb[b][:, :, :, HD:2 * HD], 1.0)
            outT_sb[b] = ot_pool.tile([P, KT, SEQ], BF16, tag="ot",
                                      name=f"ot{b}")

        # ---------------- filler units ----------------
        def qk_unit(b, slot, qt):
            ps = mm_ps.tile([P, NQ], F32, tag="mm", name=f"qk{b}_{slot}_{qt}")
            for kt in range(KT):
                nc.tensor.matmul(
                    ps,
                    lhsT=wqkv_sb[:, kt, slot * P:(slot + 1) * P],
                    rhs=xT_sb[b][:, kt, qt * NQ:(qt + 1) * NQ],
                    start=(kt == 0),
                    stop=(kt == KT - 1),
                    skip_group_check=True,
                )
            nc.vector.tensor_copy(
                out=qkT_sb[b][:, slot, qt * NQ:(qt + 1) * NQ], in_=ps
            )

        def v_unit(b, tt, nt):
            ps = mm_ps.tile([P, NQ], F32, tag="mm", name=f"v{b}_{tt}_{nt}")
            for kt in range(KT):
                nc.tensor.matmul(
                    ps[:, :NV],
                    lhsT=xT_sb[b][:, kt, tt * P:(tt + 1) * P],
                    rhs=wqkv_sb[:, kt, 2 * D + nt * NV:2 * D + (nt + 1) * NV],
                    start=(kt == 0),
                    stop=(kt == KT - 1),
                    skip_group_check=True,
                )
            nc.vector.tensor_copy(
                out=v_sb[b][:, tt, nt * 6:(nt + 1) * 6, 0:HD],
                in_=ps[:, :NV].rearrange("p (h d) -> p h d", d=HD),
            )

        def proj_unit(b, eb, qt):
            # yT[e-block, tok-chunk] = sum_d wproj[d, e]^T @ outT[d, tok]
            # dt2-th MM only reads head-pair dt2's outT slice, so the per-MM
            # deps let most of the unit run before the spine fully finishes.
            t0 = qt * NQ
            ps = mm_ps.tile([P, NQ], F32, tag="mm", name=f"p{b}_{eb}_{qt}")
            for dt2 in range(KT):
                nc.tensor.matmul(
                    ps,
                    lhsT=wproj_sb[:, dt2, eb * P:(eb + 1) * P],
                    rhs=outT_sb[b][:, dt2, t0:t0 + NQ],
                    start=(dt2 == 0),
                    stop=(dt2 == KT - 1),
                    skip_group_check=True,
                )
            y_sb = y_pool.tile([P, NQ], BF16, tag="y", name=f"y{b}_{eb}_{qt}")
            if eb % 2 == 0:
                nc.scalar.activation(
                    out=y_sb,
                    in_=ps,
                    func=mybir.ActivationFunctionType.Identity,
                    bias=bias_sb[:, eb:eb + 1],
                    scale=1.0,
                )
            else:
                nc.vector.tensor_scalar_add(
                    out=y_sb, in0=ps, scalar1=bias_sb[:, eb:eb + 1]
                )
            nc.sync.dma_start(
                out=yT_d[eb * P:(eb + 1) * P,
                         b * SEQ + t0:b * SEQ + t0 + NQ],
                in_=y_sb,
            )

        # need-order for a batch's QKV/V units (spine consumption order).
        FILL_ORDER_B0 = [
            ("qk", 0, 0), ("qk", 1, 0),
            ("v", 0, 0), ("v", 1, 0),
            ("qk", 0, 1),
            ("v", 2, 0), ("v", 3, 0),
            ("qk", 2, 0), ("qk", 3, 0), ("qk", 2, 1),
            ("v", 4, 0), ("v", 5, 0),
            ("qk", 4, 0), ("qk", 5, 0), ("qk", 4, 1),
            ("v", 6, 0), ("v", 7, 0),
            ("qk", 1, 1),
            ("qk", 6, 0), ("qk", 7, 0), ("qk", 6, 1),
            ("v", 0, 1), ("v", 1, 1), ("v", 2, 1),
            ("qk", 3, 1),
            ("qk", 8, 0), ("qk", 9, 0), ("qk", 8, 1),
            ("v", 3, 1), ("v", 4, 1),
            ("qk", 10, 0), ("qk", 11, 0), ("qk", 10, 1),
            ("qk", 5, 1),
            ("v", 5, 1), ("v", 6, 1), ("v", 7, 1),
            ("qk", 7, 1), ("qk", 9, 1), ("qk", 11, 1),
        ]
        # b1: same order, but the odd-slot (Q) qt=1 units are deferred to a
        # late band so the final spine still has backfill (see below).
        B1_LATE = {("qk", 1, 1), ("qk", 3, 1), ("qk", 5, 1),
                   ("qk", 7, 1), ("qk", 9, 1), ("qk", 11, 1)}
        FILL_ORDER_B1 = [u for u in FILL_ORDER_B0 if u not in B1_LATE]

        band(100000)
        for kind, a, c in FILL_ORDER_B0:
            (qk_unit if kind == "qk" else v_unit)(0, a, c)
        band(200000)
        for kind, a, c in FILL_ORDER_B1:
            (qk_unit if kind == "qk" else v_unit)(1, a, c)
        # late-deadline fills: needed only at spine(b1,qt1) start; banded
        # above the proj fills of earlier (b,qt) so they survive until the
        # late spines instead of being gobbled by early bubbles.
        band(325000)
        for kind, a, c in sorted(B1_LATE, key=lambda u: u[1]):
            qk_unit(1, a, c)

        # ---------------- attention spine (lowest priorities) -------------
        def attn_unit(b, qt, hp):
            avs = [
                av_ps.tile([P, NQ], F32, tag="av", name=f"av{b}_{hp}_{qt}_{i}")
                for i in range(2)
            ]
            epairs = {}
            stps = {}

            def st_pair(kt):
                stp = st_ps.tile(
                    [P, 2, NQ], F32, tag="st", name=f"st{b}_{hp}_{qt}_{kt}"
                )
                for hi in range(2):
                    base = hi * HD
                    nc.tensor.matmul(
                        stp[:, hi, :],
                        lhsT=qkT_sb[b][
                            base:base + HD, 2 * hp, kt * P:(kt + 1) * P
                        ],
                        rhs=qkT_sb[b][
                            base:base + HD, 2 * hp + 1, qt * NQ:(qt + 1) * NQ
                        ],
                        start=True,
                        stop=True,
                        skip_group_check=True,
                    )
                stps[kt] = stp

            def exp_t(kt):
                e_t = e_pool.tile(
                    [P, 2, NQ], BF16, tag="e", name=f"e{b}_{hp}_{qt}_{kt}"
                )
                nc.scalar.activation(
                    out=e_t,
                    in_=stps.pop(kt),
                    func=mybir.ActivationFunctionType.Exp,
                    scale=SCALE,
                )
                epairs[kt] = e_t

            def av(hi, kt):
                nc.tensor.matmul(
                    avs[hi],
                    lhsT=v_sb[b][:, kt, 2 * hp + hi, :],
                    rhs=epairs[kt][:, hi, :],
                    start=(kt == 0),
                    stop=(kt == KTT - 1),
                    skip_group_check=True,
                )

            def normalize(hi):
                base = hi * HD
                den = dn_pool.tile(
                    [HD, NQ], F32, tag="den", name=f"den{b}_{hp}_{qt}_{hi}"
                )
                nc.vector.tensor_copy(out=den, in_=avs[hi][HD:2 * HD, :])
                rb = rb_pool.tile(
                    [HD, NQ], F32, tag="rb", name=f"rb{b}_{hp}_{qt}_{hi}"
                )
                nc.vector.reciprocal_approx_fast(out=rb, in_=den)
                nc.vector.tensor_mul(
                    out=outT_sb[b][
                        base:base + HD, hp, qt * NQ:(qt + 1) * NQ
                    ],
                    in0=avs[hi][0:HD, :],
                    in1=rb,
                )

            st_pair(0)
            exp_t(0)
            st_pair(1)
            exp_t(1)
            for kt in range(2, KTT):
                st_pair(kt)
                exp_t(kt)
                av(0, kt - 2)
                av(1, kt - 2)
            for kt in (KTT - 2, KTT - 1):
                av(0, kt)
                av(1, kt)
            normalize(0)
            normalize(1)

        # Dependencies are tracked in EMISSION order (priorities only
        # reorder within the dep graph), so proj units — which read outT —
        # must be emitted after the spine units that write it.
        for b in range(BPC):
            for qt in range(QT):
                band(1000 + (2 * b + qt) * 1000)
                for hp in range(H // 2):
                    attn_unit(b, qt, hp)
                band(300000 + (2 * b + qt) * 10000)
                for eb in range(KT):
                    proj_unit(b, eb, qt)


def _build_program():
    nc = bacc.Bacc()
    xT_d = nc.declare_dram_parameter("xT", [D, T], BF16, isOutput=False)
    wqkv_d = nc.declare_dram_parameter("wqkv", [D, 3 * D], BF16, isOutput=False)
    wproj_d = nc.declare_dram_parameter("wproj", [D, D], BF16, isOutput=False)
    bias_d = nc.declare_dram_parameter("bias", [D], F32, isOutput=False)
    yT_d = nc.declare_dram_parameter("yT", [D, T], BF16, isOutput=True)

    with tile.TileContext(nc) as tc:
        _emit(tc, xT_d, wqkv_d, wproj_d, bias_d, yT_d)
    nc.compile()
    return nc


_NC = None


def _get_nc():
    global _NC
    if _NC is None:
        _NC = _build_program()
    return _NC


def _qk_slot_perm():
    """Column permutation for the Q|K part of qkv_w: slot 2h <- K head-pair h,
    slot 2h+1 <- Q head-pair h."""
    perm = []
    for hp in range(H // 2):
        perm.extend(range(D + hp * P, D + (hp + 1) * P))      # K slot
        perm.extend(range(hp * P, (hp + 1) * P))              # Q slot
    return np.array(perm)


def _prep_in_maps(x, qkv_w, proj_w, proj_b):
    bf16 = ml_dtypes.bfloat16
    qkv_w = np.asarray(qkv_w)
    perm = _qk_slot_perm()
    wq_perm = np.concatenate([qkv_w[:, perm], qkv_w[:, 2 * D:]], axis=1)
    wq = np.ascontiguousarray(wq_perm.astype(bf16))
    wp = np.ascontiguousarray(np.asarray(proj_w).astype(bf16))
    pb = np.ascontiguousarray(np.asarray(proj_b).astype(np.float32))
    x = np.asarray(x)
    in_maps = []
    for c in range(N_CORES):
        xc = x[c * BPC:(c + 1) * BPC].reshape(T, D).astype(bf16)
        xTc = np.ascontiguousarray(xc.T)  # [D, T] bf16
        in_maps.append({"xT": xTc, "wqkv": wq, "wproj": wp, "bias": pb})
    return in_maps


def _run(x, qkv_w, proj_w, proj_b, **spmd_kwargs):
    nc = _get_nc()
    in_maps = _prep_in_maps(x, qkv_w, proj_w, proj_b)
    res = run_bass_kernel_spmd(nc, in_maps, core_ids=list(range(N_CORES)), **spmd_kwargs)
    # yT is [D, T] per core; transpose back to [T, D] on host.
    y = np.stack(
        [res.results[c]["yT"].astype(np.float32).T for c in range(N_CORES)]
    )
    return y.reshape(B, SEQ, D), res


def kernel(x, qkv_w, proj_w, proj_b):
    y, _ = _run(x, qkv_w, proj_w, proj_b)
    return y


# revision 29
# speedup vs baseline: 1.0954x; 1.0022x over previous
"""Trainium2 Bass kernel: batched multi-head self-attention (B=16, N=1024, D=768, H=12).

Strategy
--------
Data-parallel over the batch: 16 batches / 8 NeuronCores = 2 batches per core.
Each core runs an identical (SPMD) Bass program over its shard.

Per-core math, all matmuls in bf16 with fp32 PSUM accumulation:
  * Host pre-transposes x to xT [D, T] (T = 2048 local tokens) and casts
    x / qkv_w / proj_w to bf16.  qkv_w's Q|K columns are permuted on host
    into (K0,Q0,K1,Q1,...,K5,Q5) slot order so the first DMA chunks carry
    exactly what the first attention head-pair needs.
      - Q^T,K^T [c, tok] = matmul(lhsT=wqkv[:, slot], rhs=xT)
      - V [tok, c]       = matmul(lhsT=xT[:, tok-tile], rhs=wqkv_v)
      - S^T [k, q]       = matmul(lhsT=K^T_h [hd, k-tile], rhs=Q^T_h [hd, q])
        head pairs 2i/2i+1 live at partition bases 0/64 -> row-tiles
        (0,0)/(64,0) of the 64x128 PE config.  Score pairs are emitted in
        BURSTS of 2 consecutive k-tiles (4 MMs back to back): entering and
        leaving the 64-row PE config costs ~100ns of pipeline drain each
        way, so amortizing 2 pairs per switch halves that tax.
      - exp on ScalarE straight out of PSUM, bf16 into SBUF
      - out^T [hd, q]    = matmul(lhsT=[V_h | ones(64)], rhs=expT [k, q]);
        psum rows 64-127 = softmax denominator, replicated for free.
      - normalize: copy denominator to SBUF (reciprocal_approx_fast's
        BITWISE_NOT seed needs raw fp32 bits; PSUM reads don't deliver
        those on HW), reciprocal, tensor_mul -> outT (bf16)
      - y^T [e, tok]     = matmul(lhsT=proj_w[:, e-block], rhs=out^T);
        ScalarE ACT (Identity + per-partition bias) drains PSUM -> bf16
        SBUF, so the projection tail never queues behind the DVE.  The
        host transposes y^T back to [tok, e] when unsharding.

Scheduling: the Tile framework scheduler is dependency-driven with a
priority heap per engine, so ordering is controlled via priority BANDS:
the attention spine (S^T bursts, exp, A@V, normalize) gets the lowest
priority numbers and preempts whenever its dependencies are satisfied;
QKV / V / proj matmul units sit in higher bands and automatically
backfill every PE bubble.  Fill bands are deadline-ordered (late-needed
units get later bands) so the final spine doesn't run dry of backfill.
DMA chunks are need-ordered at fine grain (x in qt-halves, weights in
slot-pair columns) so the PE starts ~1us in and stays busy, warming the
HAM clock gate early.

kernel() takes full unsharded inputs, shards on host, runs all 8 cores via
run_bass_kernel_spmd, and re-assembles the full output.
"""

import numpy as np
import ml_dtypes

import concourse.bass as bass
import concourse.mybir as mybir
import concourse.tile as tile
from concourse import bacc
from concourse.bass_utils import run_bass_kernel_spmd

BF16 = mybir.dt.bfloat16
F32 = mybir.dt.float32

N_CORES = 8
B, SEQ, D = 16, 1024, 768
H, HD = 12, 64
BPC = B // N_CORES            # batches per core
T = BPC * SEQ                 # tokens per core
P = 128
KT = D // P                   # 6 contraction sub-tiles of 128
NQ = 512                      # moving free-dim per matmul (1 psum bank of fp32)
QT = SEQ // NQ                # 2 query tiles per batch
KTT = SEQ // P                # 8 key-token tiles per batch
NV = 384                      # V-projection output tile (2 per 768)
SCALE = HD ** -0.5


def _emit(tc, xT_d, wqkv_d, wproj_d, bias_d, yT_d):
    nc = tc.nc
    from contextlib import ExitStack

    def band(n):
        tc.cur_priority = n

    with ExitStack() as ctx:
        consts = ctx.enter_context(tc.tile_pool(name="consts", bufs=1))
        xt_pool = ctx.enter_context(tc.tile_pool(name="xt", bufs=2))
        qk_pool = ctx.enter_context(tc.tile_pool(name="qkT", bufs=2))
        v_pool = ctx.enter_context(tc.tile_pool(name="v", bufs=2))
        ot_pool = ctx.enter_context(tc.tile_pool(name="ot", bufs=2))
        e_pool = ctx.enter_context(tc.tile_pool(name="e", bufs=5))
        dn_pool = ctx.enter_context(tc.tile_pool(name="dn", bufs=2))
        rb_pool = ctx.enter_context(tc.tile_pool(name="rb", bufs=2))
        y_pool = ctx.enter_context(tc.tile_pool(name="y", bufs=8))
        mm_ps = ctx.enter_context(tc.tile_pool(name="mmps", bufs=2, space="PSUM"))
        st_ps = ctx.enter_context(tc.tile_pool(name="stps", bufs=2, space="PSUM"))
        av_ps = ctx.enter_context(tc.tile_pool(name="avps", bufs=2, space="PSUM"))

        # ---------------- DMA (need-ordered chunks), band 0 ----------------
        band(0)
        xT_full = xT_d[:].rearrange("(po pi) t -> pi po t", pi=P)   # [128, 6, T]
        wqkv_full = wqkv_d[:].rearrange("(po pi) c -> pi po c", pi=P)

        wqkv_sb = consts.tile([P, KT, 3 * D], BF16)
        xT_sb = {}
        for b in range(BPC):
            xT_sb[b] = xt_pool.tile([P, KT, SEQ], BF16, tag="xt", name=f"xT{b}")

        # Weights issue on the Scalar HWDGE queue, x on the Sync queue: the
        # ~600ns-per-dma_start descriptor-generation cost then runs on two
        # queues in parallel, so the first qk unit's inputs land ~2us
        # earlier.  Chunk sizes balance issue cost vs need-granularity.
        def dma_w(c0, c1):
            nc.scalar.dma_start(out=wqkv_sb[:, :, c0:c1],
                                in_=wqkv_full[:, :, c0:c1])

        def dma_x(b, k0, k1, t0, t1):
            nc.sync.dma_start(out=xT_sb[b][:, k0:k1, t0:t1],
                              in_=xT_full[:, k0:k1, b * SEQ + t0:b * SEQ + t1])

        dma_w(0, 256)                  # hp0's K|Q slots
        for k0 in range(0, KT, 2):     # qt0 half of x(b0), 2-kt chunks
            dma_x(0, k0, k0 + 2, 0, NQ)
        dma_w(2 * D, 2 * D + NV)       # V cols nt=0 (heads 0..5)
        for k0 in range(0, KT, 2):     # qt1 half (K qt1 + v token-tiles 4-7)
            dma_x(0, k0, k0 + 2, NQ, SEQ)
        dma_w(256, 768)                # slot-pairs 1,2
        dma_x(1, 0, KT, 0, NQ)
        dma_w(768, 2 * D)              # slot-pairs 3,4,5
        dma_w(2 * D + NV, 3 * D)       # V cols nt=1 (first needed by hp3)
        dma_x(1, 0, KT, NQ, SEQ)
        wproj_sb = consts.tile([P, KT, D], BF16)
        nc.scalar.dma_start(
            out=wproj_sb, in_=wproj_d[:].rearrange("(po pi) c -> pi po c", pi=P)
        )
        # bias laid out per-partition for the yT drain: bias_sb[p, e] =
        # bias[e*128 + p]
        bias_sb = consts.tile([P, KT], F32)
        nc.scalar.dma_start(
            out=bias_sb, in_=bias_d[:].rearrange("(e p) -> p e", p=P)
        )

        # HAM warm-up: the PE clock-gate defaults to half rate and needs
        # ~3.4us of sustained matmul activity to lift; a dozen garbage
        # matmuls on a zeroed scratch tile flip it before the first real
        # unit's inputs even arrive (results are never read).
        ws = consts.tile([P, NQ], BF16)
        nc.vector.memset(ws, 0.0)
        for w_i in range(20):
            wps = mm_ps.tile([P, NQ], F32, tag="mm", name=f"warm{w_i}")
            nc.tensor.matmul(
                wps[:, 0:2 * P],
                lhsT=ws[:, 0:P],
                rhs=ws[:, 0:2 * P],
                start=True,
                stop=True,
                skip_group_check=True,
            )

        # ---------------- per-batch state ----------------
        qkT_sb, v_sb, outT_sb = {}, {}, {}
        for b in range(BPC):
            qkT_sb[b] = qk_pool.tile([P, 2 * KT, SEQ], BF16, tag="qkT",
                                     name=f"qkT{b}")
            v_sb[b] = v_pool.tile([P, KTT, H, 2 * HD], BF16, tag="v",
                                  name=f"v{b}")
            nc.gpsimd.memset(v_sb[b][:, :, :, HD:2 * HD], 1.0)
            outT_sb[b] = ot_pool.tile([P, KT, SEQ], BF16, tag="ot",
                                      name=f"ot{b}")

        # ---------------- filler units ----------------
        def qk_unit(b, slot, qt):
            ps = mm_ps.tile([P, NQ], F32, tag="mm", name=f"qk{b}_{slot}_{qt}")
            for kt in range(KT):
                nc.tensor.matmul(
                    ps,
                    lhsT=wqkv_sb[:, kt, slot * P:(slot + 1) * P],
                    rhs=xT_sb[b][:, kt, qt * NQ:(qt + 1) * NQ],
                    start=(kt == 0),
                    stop=(kt == KT - 1),
                    skip_group_check=True,
                )
            nc.vector.tensor_copy(
                out=qkT_sb[b][:, slot, qt * NQ:(qt + 1) * NQ], in_=ps
            )

        def v_unit(b, tt, nt):
            ps = mm_ps.tile([P, NQ], F32, tag="mm", name=f"v{b}_{tt}_{nt}")
            for kt in range(KT):
                nc.tensor.matmul(
                    ps[:, :NV],
                    lhsT=xT_sb[b][:, kt, tt * P:(tt + 1) * P],
                    rhs=wqkv_sb[:, kt, 2 * D + nt * NV:2 * D + (nt + 1) * NV],
                    start=(kt == 0),
                    stop=(kt == KT - 1),
                    skip_group_check=True,
                )
            nc.vector.tensor_copy(
                out=v_sb[b][:, tt, nt * 6:(nt + 1) * 6, 0:HD],
                in_=ps[:, :NV].rearrange("p (h d) -> p h d", d=HD),
            )

        def proj_unit(b, eb, qt, last=False):
            # yT[e-block, tok-chunk] = sum_d wproj[d, e]^T @ outT[d, tok]
            # For the final (b,qt) the attention-spine PSUM banks are free,
            # so alternating proj units onto the av pool deepens the drain
            # pipeline right where the kernel tail forms.
            t0 = qt * NQ
            pool = av_ps if (last and eb % 2 == 1) else mm_ps
            ps = pool.tile([P, NQ], F32, tag="av" if pool is av_ps else "mm",
                           name=f"p{b}_{eb}_{qt}")
            for dt2 in range(KT):
                nc.tensor.matmul(
                    ps,
                    lhsT=wproj_sb[:, dt2, eb * P:(eb + 1) * P],
                    rhs=outT_sb[b][:, dt2, t0:t0 + NQ],
                    start=(dt2 == 0),
                    stop=(dt2 == KT - 1),
                    skip_group_check=True,
                )
            y_sb = y_pool.tile([P, NQ], BF16, tag="y", name=f"y{b}_{eb}_{qt}")
            if eb % 2 == 0:
                nc.scalar.activation(
                    out=y_sb,
                    in_=ps,
                    func=mybir.ActivationFunctionType.Identity,
                    bias=bias_sb[:, eb:eb + 1],
                    scale=1.0,
                )
            else:
                nc.vector.tensor_scalar_add(
                    out=y_sb, in0=ps, scalar1=bias_sb[:, eb:eb + 1]
                )
            if last:
                # split the tail writes across both HWDGE queues so the
                # final drain chain isn't serialized on one queue's issue.
                nc.sync.dma_start(
                    out=yT_d[eb * P:(eb + 1) * P,
                             b * SEQ + t0:b * SEQ + t0 + NQ // 2],
                    in_=y_sb[:, 0:NQ // 2],
                )
                nc.scalar.dma_start(
                    out=yT_d[eb * P:(eb + 1) * P,
                             b * SEQ + t0 + NQ // 2:b * SEQ + t0 + NQ],
                    in_=y_sb[:, NQ // 2:NQ],
                )
            else:
                nc.sync.dma_start(
                    out=yT_d[eb * P:(eb + 1) * P,
                             b * SEQ + t0:b * SEQ + t0 + NQ],
                    in_=y_sb,
                )

        # need-order for a batch's QKV/V units (spine consumption order).
        FILL_ORDER_B0 = [
            ("qk", 0, 0), ("qk", 1, 0),
            ("v", 0, 0), ("v", 1, 0),
            ("qk", 0, 1),
            ("v", 2, 0), ("v", 3, 0),
            ("qk", 2, 0), ("qk", 3, 0), ("qk", 2, 1),
            ("v", 4, 0), ("v", 5, 0),
            ("qk", 4, 0), ("qk", 5, 0), ("qk", 4, 1),
            ("v", 6, 0), ("v", 7, 0),
            ("qk", 1, 1),
            ("qk", 6, 0), ("qk", 7, 0), ("qk", 6, 1),
            ("v", 0, 1), ("v", 1, 1), ("v", 2, 1),
            ("qk", 3, 1),
            ("qk", 8, 0), ("qk", 9, 0), ("qk", 8, 1),
            ("v", 3, 1), ("v", 4, 1),
            ("qk", 10, 0), ("qk", 11, 0), ("qk", 10, 1),
            ("qk", 5, 1),
            ("v", 5, 1), ("v", 6, 1), ("v", 7, 1),
            ("qk", 7, 1), ("qk", 9, 1), ("qk", 11, 1),
        ]
        # b1: same order, but the odd-slot (Q) qt=1 units are deferred to a
        # late band so the final spine still has backfill (see below).
        B1_LATE = {("qk", 1, 1), ("qk", 3, 1), ("qk", 5, 1),
                   ("qk", 7, 1), ("qk", 9, 1), ("qk", 11, 1)}
        FILL_ORDER_B1 = [u for u in FILL_ORDER_B0 if u not in B1_LATE]

        band(100000)
        for kind, a, c in FILL_ORDER_B0:
            (qk_unit if kind == "qk" else v_unit)(0, a, c)
        band(200000)
        for kind, a, c in FILL_ORDER_B1:
            (qk_unit if kind == "qk" else v_unit)(1, a, c)
        # late-deadline fills: needed only at spine(b1,qt1) start; banded
        # above the proj fills of earlier (b,qt) so they survive until the
        # late spines instead of being gobbled by early bubbles.
        band(325000)
        for kind, a, c in sorted(B1_LATE, key=lambda u: u[1]):
            qk_unit(1, a, c)

        # ---------------- attention spine (lowest priorities) -------------
        def attn_unit(b, qt, hp):
            avs = [
                av_ps.tile([P, NQ], F32, tag="av", name=f"av{b}_{hp}_{qt}_{i}")
                for i in range(2)
            ]
            epairs = {}
            stps = {}

            def st_pair(kt):
                stp = st_ps.tile(
                    [P, 2, NQ], F32, tag="st", name=f"st{b}_{hp}_{qt}_{kt}"
                )
                for hi in range(2):
                    base = hi * HD
                    nc.tensor.matmul(
                        stp[:, hi, :],
                        lhsT=qkT_sb[b][
                            base:base + HD, 2 * hp, kt * P:(kt + 1) * P
                        ],
                        rhs=qkT_sb[b][
                            base:base + HD, 2 * hp + 1, qt * NQ:(qt + 1) * NQ
                        ],
                        start=True,
                        stop=True,
                        skip_group_check=True,
                    )
                stps[kt] = stp

            def exp_t(kt):
                e_t = e_pool.tile(
                    [P, 2, NQ], BF16, tag="e", name=f"e{b}_{hp}_{qt}_{kt}"
                )
                nc.scalar.activation(
                    out=e_t,
                    in_=stps.pop(kt),
                    func=mybir.ActivationFunctionType.Exp,
                    scale=SCALE,
                )
                epairs[kt] = e_t

            def av(hi, kt):
                nc.tensor.matmul(
                    avs[hi],
                    lhsT=v_sb[b][:, kt, 2 * hp + hi, :],
                    rhs=epairs[kt][:, hi, :],
                    start=(kt == 0),
                    stop=(kt == KTT - 1),
                    skip_group_check=True,
                )

            def normalize(hi):
                base = hi * HD
                den = dn_pool.tile(
                    [HD, NQ], F32, tag="den", name=f"den{b}_{hp}_{qt}_{hi}"
                )
                nc.vector.tensor_copy(out=den, in_=avs[hi][HD:2 * HD, :])
                rb = rb_pool.tile(
                    [HD, NQ], F32, tag="rb", name=f"rb{b}_{hp}_{qt}_{hi}"
                )
                nc.vector.reciprocal_approx_fast(out=rb, in_=den)
                nc.vector.tensor_mul(
                    out=outT_sb[b][
                        base:base + HD, hp, qt * NQ:(qt + 1) * NQ
                    ],
                    in0=avs[hi][0:HD, :],
                    in1=rb,
                )

            st_pair(0)
            exp_t(0)
            st_pair(1)
            exp_t(1)
            for kt in range(2, KTT):
                st_pair(kt)
                exp_t(kt)
                av(0, kt - 2)
                av(1, kt - 2)
            for kt in (KTT - 2, KTT - 1):
                av(0, kt)
                av(1, kt)
            normalize(0)
            normalize(1)

        # Dependencies are tracked in EMISSION order (priorities only
        # reorder within the dep graph), so proj units — which read outT —
        # must be emitted after the spine units that write it.
        for b in range(BPC):
            for qt in range(QT):
                band(1000 + (2 * b + qt) * 1000)
                for hp in range(H // 2):
                    attn_unit(b, qt, hp)
                band(300000 + (2 * b + qt) * 10000)
                last = (b == BPC - 1 and qt == QT - 1)
                for eb in range(KT):
                    proj_unit(b, eb, qt, last=last)


def _build_program():
    nc = bacc.Bacc()
    xT_d = nc.declare_dram_parameter("xT", [D, T], BF16, isOutput=False)
    wqkv_d = nc.declare_dram_parameter("wqkv", [D, 3 * D], BF16, isOutput=False)
    wproj_d = nc.declare_dram_parameter("wproj", [D, D], BF16, isOutput=False)
    bias_d = nc.declare_dram_parameter("bias", [D], F32, isOutput=False)
    yT_d = nc.declare_dram_parameter("yT", [D, T], BF16, isOutput=True)

    with tile.TileContext(nc) as tc:
        _emit(tc, xT_d, wqkv_d, wproj_d, bias_d, yT_d)
    nc.compile()
    return nc


_NC = None


def _get_nc():
    global _NC
    if _NC is None:
        _NC = _build_program()
    return _NC


def _qk_slot_perm():
    """Column permutation for the Q|K part of qkv_w: slot 2h <- K head-pair h,
    slot 2h+1 <- Q head-pair h."""
    perm = []
    for hp in range(H // 2):
        perm.extend(range(D + hp * P, D + (hp + 1) * P))      # K slot
        perm.extend(range(hp * P, (hp + 1) * P))              # Q slot
    return np.array(perm)


def _prep_in_maps(x, qkv_w, proj_w, proj_b):
    bf16 = ml_dtypes.bfloat16
    qkv_w = np.asarray(qkv_w)
    perm = _qk_slot_perm()
    wq_perm = np.concatenate([qkv_w[:, perm], qkv_w[:, 2 * D:]], axis=1)
    wq = np.ascontiguousarray(wq_perm.astype(bf16))
    wp = np.ascontiguousarray(np.asarray(proj_w).astype(bf16))
    pb = np.ascontiguousarray(np.asarray(proj_b).astype(np.float32))
    x = np.asarray(x)
    in_maps = []
    for c in range(N_CORES):
        xc = x[c * BPC:(c + 1) * BPC].reshape(T, D).astype(bf16)
        xTc = np.ascontiguousarray(xc.T)  # [D, T] bf16
        in_maps.append({"xT": xTc, "wqkv": wq, "wproj": wp, "bias": pb})
    return in_maps


def _run(x, qkv_w, proj_w, proj_b, **spmd_kwargs):
    nc = _get_nc()
    in_maps = _prep_in_maps(x, qkv_w, proj_w, proj_b)
    res = run_bass_kernel_spmd(nc, in_maps, core_ids=list(range(N_CORES)), **spmd_kwargs)
    # yT is [D, T] per core; transpose back to [T, D] on host.
    y = np.stack(
        [res.results[c]["yT"].astype(np.float32).T for c in range(N_CORES)]
    )
    return y.reshape(B, SEQ, D), res


def kernel(x, qkv_w, proj_w, proj_b):
    y, _ = _run(x, qkv_w, proj_w, proj_b)
    return y


# revision 32
# speedup vs baseline: 1.1034x; 1.0074x over previous
"""Trainium2 Bass kernel: batched multi-head self-attention (B=16, N=1024, D=768, H=12).

Strategy
--------
Data-parallel over the batch: 16 batches / 8 NeuronCores = 2 batches per core.
Each core runs an identical (SPMD) Bass program over its shard.

Per-core math, all matmuls in bf16 with fp32 PSUM accumulation:
  * Host pre-transposes x to xT [D, T] (T = 2048 local tokens) and casts
    x / qkv_w / proj_w to bf16.  qkv_w's Q|K columns are permuted on host
    into (K0,Q0,K1,Q1,...,K5,Q5) slot order so the first DMA chunks carry
    exactly what the first attention head-pair needs.
      - Q^T,K^T [c, tok] = matmul(lhsT=wqkv[:, slot], rhs=xT)
      - V [tok, c]       = matmul(lhsT=xT[:, tok-tile], rhs=wqkv_v)
      - S^T [k, q]       = matmul(lhsT=K^T_h [hd, k-tile], rhs=Q^T_h [hd, q])
        head pairs 2i/2i+1 live at partition bases 0/64 -> row-tiles
        (0,0)/(64,0) of the 64x128 PE config.  Score pairs are emitted in
        BURSTS of 2 consecutive k-tiles (4 MMs back to back): entering and
        leaving the 64-row PE config costs ~100ns of pipeline drain each
        way, so amortizing 2 pairs per switch halves that tax.
      - exp on ScalarE straight out of PSUM, bf16 into SBUF
      - out^T [hd, q]    = matmul(lhsT=[V_h | ones(64)], rhs=expT [k, q]);
        psum rows 64-127 = softmax denominator, replicated for free.
      - normalize: copy denominator to SBUF (reciprocal_approx_fast's
        BITWISE_NOT seed needs raw fp32 bits; PSUM reads don't deliver
        those on HW), reciprocal, tensor_mul -> outT (bf16)
      - y^T [e, tok]     = matmul(lhsT=proj_w[:, e-block], rhs=out^T);
        ScalarE ACT (Identity + per-partition bias) drains PSUM -> bf16
        SBUF, so the projection tail never queues behind the DVE.  The
        host transposes y^T back to [tok, e] when unsharding.

Scheduling: the Tile framework scheduler is dependency-driven with a
priority heap per engine, so ordering is controlled via priority BANDS:
the attention spine (S^T bursts, exp, A@V, normalize) gets the lowest
priority numbers and preempts whenever its dependencies are satisfied;
QKV / V / proj matmul units sit in higher bands and automatically
backfill every PE bubble.  Fill bands are deadline-ordered (late-needed
units get later bands) so the final spine doesn't run dry of backfill.
DMA chunks are need-ordered at fine grain (x in qt-halves, weights in
slot-pair columns) so the PE starts ~1us in and stays busy, warming the
HAM clock gate early.

kernel() takes full unsharded inputs, shards on host, runs all 8 cores via
run_bass_kernel_spmd, and re-assembles the full output.
"""

import numpy as np
import ml_dtypes

import concourse.bass as bass
import concourse.mybir as mybir
import concourse.tile as tile
from concourse import bacc
from concourse.bass_utils import run_bass_kernel_spmd

BF16 = mybir.dt.bfloat16
F32 = mybir.dt.float32

N_CORES = 8
B, SEQ, D = 16, 1024, 768
H, HD = 12, 64
BPC = B // N_CORES            # batches per core
T = BPC * SEQ                 # tokens per core
P = 128
KT = D // P                   # 6 contraction sub-tiles of 128
NQ = 512                      # moving free-dim per matmul (1 psum bank of fp32)
QT = SEQ // NQ                # 2 query tiles per batch
KTT = SEQ // P                # 8 key-token tiles per batch
NV = 384                      # V-projection output tile (2 per 768)
SCALE = HD ** -0.5


def _emit(tc, xT_d, wqkv_d, wproj_d, bias_d, yT_d):
    nc = tc.nc
    from contextlib import ExitStack

    def band(n):
        tc.cur_priority = n

    with ExitStack() as ctx:
        consts = ctx.enter_context(tc.tile_pool(name="consts", bufs=1))
        xt_pool = ctx.enter_context(tc.tile_pool(name="xt", bufs=2))
        qk_pool = ctx.enter_context(tc.tile_pool(name="qkT", bufs=2))
        v_pool = ctx.enter_context(tc.tile_pool(name="v", bufs=2))
        ot_pool = ctx.enter_context(tc.tile_pool(name="ot", bufs=2))
        e_pool = ctx.enter_context(tc.tile_pool(name="e", bufs=5))
        dn_pool = ctx.enter_context(tc.tile_pool(name="dn", bufs=2))
        rb_pool = ctx.enter_context(tc.tile_pool(name="rb", bufs=2))
        y_pool = ctx.enter_context(tc.tile_pool(name="y", bufs=8))
        mm_ps = ctx.enter_context(tc.tile_pool(name="mmps", bufs=2, space="PSUM"))
        st_ps = ctx.enter_context(tc.tile_pool(name="stps", bufs=2, space="PSUM"))
        av_ps = ctx.enter_context(tc.tile_pool(name="avps", bufs=2, space="PSUM"))

        # ---------------- DMA (need-ordered chunks), band 0 ----------------
        band(0)
        xT_full = xT_d[:].rearrange("(po pi) t -> pi po t", pi=P)   # [128, 6, T]
        wqkv_full = wqkv_d[:].rearrange("(po pi) c -> pi po c", pi=P)

        wqkv_sb = consts.tile([P, KT, 3 * D], BF16)
        xT_sb = {}
        for b in range(BPC):
            xT_sb[b] = xt_pool.tile([P, KT, SEQ], BF16, tag="xt", name=f"xT{b}")

        # Weights issue on the Scalar HWDGE queue, x on the Sync queue: the
        # ~600ns-per-dma_start descriptor-generation cost then runs on two
        # queues in parallel, so the first qk unit's inputs land ~2us
        # earlier.  Chunk sizes balance issue cost vs need-granularity.
        def dma_w(c0, c1):
            nc.scalar.dma_start(out=wqkv_sb[:, :, c0:c1],
                                in_=wqkv_full[:, :, c0:c1])

        def dma_x(b, k0, k1, t0, t1):
            nc.sync.dma_start(out=xT_sb[b][:, k0:k1, t0:t1],
                              in_=xT_full[:, k0:k1, b * SEQ + t0:b * SEQ + t1])

        dma_w(0, 256)                  # hp0's K|Q slots
        for k0 in range(0, KT, 2):     # qt0 half of x(b0), 2-kt chunks
            dma_x(0, k0, k0 + 2, 0, NQ)
        dma_w(2 * D, 2 * D + NV)       # V cols nt=0 (heads 0..5)
        for k0 in range(0, KT, 2):     # qt1 half (K qt1 + v token-tiles 4-7)
            dma_x(0, k0, k0 + 2, NQ, SEQ)
        dma_w(256, 768)                # slot-pairs 1,2
        dma_x(1, 0, KT, 0, NQ)
        dma_w(768, 2 * D)              # slot-pairs 3,4,5
        dma_w(2 * D + NV, 3 * D)       # V cols nt=1 (first needed by hp3)
        dma_x(1, 0, KT, NQ, SEQ)
        wproj_sb = consts.tile([P, KT, D], BF16)
        nc.scalar.dma_start(
            out=wproj_sb, in_=wproj_d[:].rearrange("(po pi) c -> pi po c", pi=P)
        )
        # bias laid out per-partition for the yT drain: bias_sb[p, e] =
        # bias[e*128 + p]
        bias_sb = consts.tile([P, KT], F32)
        nc.scalar.dma_start(
            out=bias_sb, in_=bias_d[:].rearrange("(e p) -> p e", p=P)
        )

        # HAM warm-up: the PE clock-gate defaults to half rate and needs
        # ~3.4us of sustained matmul activity to lift; a dozen garbage
        # matmuls on a zeroed scratch tile flip it before the first real
        # unit's inputs even arrive (results are never read).
        ws = consts.tile([P, NQ], BF16)
        nc.vector.memset(ws, 0.0)
        for w_i in range(34):
            wps = mm_ps.tile([P, NQ], F32, tag="mm", name=f"warm{w_i}")
            nc.tensor.matmul(
                wps[:, 0:2 * P],
                lhsT=ws[:, 0:P],
                rhs=ws[:, 0:2 * P],
                start=True,
                stop=True,
                skip_group_check=True,
            )

        # ---------------- per-batch state ----------------
        qkT_sb, v_sb, outT_sb = {}, {}, {}
        for b in range(BPC):
            qkT_sb[b] = qk_pool.tile([P, 2 * KT, SEQ], BF16, tag="qkT",
                                     name=f"qkT{b}")
            v_sb[b] = v_pool.tile([P, KTT, H, 2 * HD], BF16, tag="v",
                                  name=f"v{b}")
            nc.gpsimd.memset(v_sb[b][:, :, :, HD:2 * HD], 1.0)
            outT_sb[b] = ot_pool.tile([P, KT, SEQ], BF16, tag="ot",
                                      name=f"ot{b}")

        # ---------------- filler units ----------------
        def qk_unit(b, slot, qt):
            ps = mm_ps.tile([P, NQ], F32, tag="mm", name=f"qk{b}_{slot}_{qt}")
            for kt in range(KT):
                nc.tensor.matmul(
                    ps,
                    lhsT=wqkv_sb[:, kt, slot * P:(slot + 1) * P],
                    rhs=xT_sb[b][:, kt, qt * NQ:(qt + 1) * NQ],
                    start=(kt == 0),
                    stop=(kt == KT - 1),
                    skip_group_check=True,
                )
            nc.vector.tensor_copy(
                out=qkT_sb[b][:, slot, qt * NQ:(qt + 1) * NQ], in_=ps
            )

        def v_unit(b, tt, nt):
            ps = mm_ps.tile([P, NQ], F32, tag="mm", name=f"v{b}_{tt}_{nt}")
            for kt in range(KT):
                nc.tensor.matmul(
                    ps[:, :NV],
                    lhsT=xT_sb[b][:, kt, tt * P:(tt + 1) * P],
                    rhs=wqkv_sb[:, kt, 2 * D + nt * NV:2 * D + (nt + 1) * NV],
                    start=(kt == 0),
                    stop=(kt == KT - 1),
                    skip_group_check=True,
                )
            nc.vector.tensor_copy(
                out=v_sb[b][:, tt, nt * 6:(nt + 1) * 6, 0:HD],
                in_=ps[:, :NV].rearrange("p (h d) -> p h d", d=HD),
            )

        def proj_unit(b, eb, qt, last=False):
            # yT[e-block, tok-chunk] = sum_d wproj[d, e]^T @ outT[d, tok]
            # For the final (b,qt): half-width token chunks + psum from the
            # (now free) attention-spine av pool + drains alternating across
            # ScalarE/DVE + output DMA split across both HWDGE queues, so
            # the end-of-kernel drain chain pipelines instead of serializing.
            t0 = qt * NQ
            nw = NQ // 2 if last else NQ
            for ci in range(NQ // nw):
                c0 = t0 + ci * nw
                pool = av_ps if (last and (2 * eb + ci) % 2 == 1) else mm_ps
                ps = pool.tile([P, NQ], F32,
                               tag="av" if pool is av_ps else "mm",
                               name=f"p{b}_{eb}_{qt}_{ci}")
                for dt2 in range(KT):
                    nc.tensor.matmul(
                        ps[:, :nw],
                        lhsT=wproj_sb[:, dt2, eb * P:(eb + 1) * P],
                        rhs=outT_sb[b][:, dt2, c0:c0 + nw],
                        start=(dt2 == 0),
                        stop=(dt2 == KT - 1),
                        skip_group_check=True,
                    )
                y_sb = y_pool.tile([P, nw], BF16, tag="y",
                                   name=f"y{b}_{eb}_{qt}_{ci}")
                use_scalar = ((2 * eb + ci) % 2 == 0) if last else (eb % 3 == 0)
                if use_scalar:
                    nc.scalar.activation(
                        out=y_sb,
                        in_=ps[:, :nw],
                        func=mybir.ActivationFunctionType.Identity,
                        bias=bias_sb[:, eb:eb + 1],
                        scale=1.0,
                    )
                else:
                    nc.vector.tensor_scalar_add(
                        out=y_sb, in0=ps[:, :nw],
                        scalar1=bias_sb[:, eb:eb + 1]
                    )
                dma_eng = nc.scalar if (last and ci % 2 == 1) else nc.sync
                dma_eng.dma_start(
                    out=yT_d[eb * P:(eb + 1) * P,
                             b * SEQ + c0:b * SEQ + c0 + nw],
                    in_=y_sb,
                )

        # need-order for a batch's QKV/V units (spine consumption order).
        FILL_ORDER_B0 = [
            ("qk", 0, 0), ("qk", 1, 0),
            ("v", 0, 0), ("v", 1, 0),
            ("qk", 0, 1),
            ("v", 2, 0), ("v", 3, 0),
            ("qk", 2, 0), ("qk", 3, 0), ("qk", 2, 1),
            ("v", 4, 0), ("v", 5, 0),
            ("qk", 4, 0), ("qk", 5, 0), ("qk", 4, 1),
            ("v", 6, 0), ("v", 7, 0),
            ("qk", 1, 1),
            ("qk", 6, 0), ("qk", 7, 0), ("qk", 6, 1),
            ("v", 0, 1), ("v", 1, 1), ("v", 2, 1),
            ("qk", 3, 1),
            ("qk", 8, 0), ("qk", 9, 0), ("qk", 8, 1),
            ("v", 3, 1), ("v", 4, 1),
            ("qk", 10, 0), ("qk", 11, 0), ("qk", 10, 1),
            ("qk", 5, 1),
            ("v", 5, 1), ("v", 6, 1), ("v", 7, 1),
            ("qk", 7, 1), ("qk", 9, 1), ("qk", 11, 1),
        ]
        # b1: same order, but the odd-slot (Q) qt=1 units are deferred to a
        # late band so the final spine still has backfill (see below).
        B1_LATE = {("qk", 1, 1), ("qk", 3, 1), ("qk", 5, 1),
                   ("qk", 7, 1), ("qk", 9, 1), ("qk", 11, 1)}
        FILL_ORDER_B1 = [u for u in FILL_ORDER_B0 if u not in B1_LATE]

        band(100000)
        for kind, a, c in FILL_ORDER_B0:
            (qk_unit if kind == "qk" else v_unit)(0, a, c)
        band(200000)
        for kind, a, c in FILL_ORDER_B1:
            (qk_unit if kind == "qk" else v_unit)(1, a, c)
        # late-deadline fills: needed only at spine(b1,qt1) start; banded
        # above the proj fills of earlier (b,qt) so they survive until the
        # late spines instead of being gobbled by early bubbles.
        band(325000)
        for kind, a, c in sorted(B1_LATE, key=lambda u: u[1]):
            qk_unit(1, a, c)

        # ---------------- attention spine (lowest priorities) -------------
        def attn_unit(b, qt, hp):
            avs = [
                av_ps.tile([P, NQ], F32, tag="av", name=f"av{b}_{hp}_{qt}_{i}")
                for i in range(2)
            ]
            epairs = {}
            stps = {}

            def st_pair(kt):
                stp = st_ps.tile(
                    [P, 2, NQ], F32, tag="st", name=f"st{b}_{hp}_{qt}_{kt}"
                )
                for hi in range(2):
                    base = hi * HD
                    nc.tensor.matmul(
                        stp[:, hi, :],
                        lhsT=qkT_sb[b][
                            base:base + HD, 2 * hp, kt * P:(kt + 1) * P
                        ],
                        rhs=qkT_sb[b][
                            base:base + HD, 2 * hp + 1, qt * NQ:(qt + 1) * NQ
                        ],
                        start=True,
                        stop=True,
                        skip_group_check=True,
                    )
                stps[kt] = stp

            def exp_t(kt):
                e_t = e_pool.tile(
                    [P, 2, NQ], BF16, tag="e", name=f"e{b}_{hp}_{qt}_{kt}"
                )
                nc.scalar.activation(
                    out=e_t,
                    in_=stps.pop(kt),
                    func=mybir.ActivationFunctionType.Exp,
                    scale=SCALE,
                )
                epairs[kt] = e_t

            def av(hi, kt):
                nc.tensor.matmul(
                    avs[hi],
                    lhsT=v_sb[b][:, kt, 2 * hp + hi, :],
                    rhs=epairs[kt][:, hi, :],
                    start=(kt == 0),
                    stop=(kt == KTT - 1),
                    skip_group_check=True,
                )

            def normalize(hi):
                base = hi * HD
                den = dn_pool.tile(
                    [HD, NQ], F32, tag="den", name=f"den{b}_{hp}_{qt}_{hi}"
                )
                nc.vector.tensor_copy(out=den, in_=avs[hi][HD:2 * HD, :])
                rb = rb_pool.tile(
                    [HD, NQ], F32, tag="rb", name=f"rb{b}_{hp}_{qt}_{hi}"
                )
                nc.vector.reciprocal_approx_fast(out=rb, in_=den)
                nc.vector.tensor_mul(
                    out=outT_sb[b][
                        base:base + HD, hp, qt * NQ:(qt + 1) * NQ
                    ],
                    in0=avs[hi][0:HD, :],
                    in1=rb,
                )

            st_pair(0)
            exp_t(0)
            st_pair(1)
            exp_t(1)
            for kt in range(2, KTT):
                st_pair(kt)
                exp_t(kt)
                av(0, kt - 2)
                av(1, kt - 2)
            for kt in (KTT - 2, KTT - 1):
                av(0, kt)
                av(1, kt)
            normalize(0)
            normalize(1)

        # Dependencies are tracked in EMISSION order (priorities only
        # reorder within the dep graph), so proj units — which read outT —
        # must be emitted after the spine units that write it.
        for b in range(BPC):
            for qt in range(QT):
                band(1000 + (2 * b + qt) * 1000)
                for hp in range(H // 2):
                    attn_unit(b, qt, hp)
                band(300000 + (2 * b + qt) * 10000)
                last = (b == BPC - 1 and qt == QT - 1)
                for eb in range(KT):
                    proj_unit(b, eb, qt, last=last)


def _build_program():
    nc = bacc.Bacc()
    xT_d = nc.declare_dram_parameter("xT", [D, T], BF16, isOutput=False)
    wqkv_d = nc.declare_dram_parameter("wqkv", [D, 3 * D], BF16, isOutput=False)
    wproj_d = nc.declare_dram_parameter("wproj", [D, D], BF16, isOutput=False)
    bias_d = nc.declare_dram_parameter("bias", [D], F32, isOutput=False)
    yT_d = nc.declare_dram_parameter("yT", [D, T], BF16, isOutput=True)

    with tile.TileContext(nc) as tc:
        _emit(tc, xT_d, wqkv_d, wproj_d, bias_d, yT_d)
    nc.compile()
    return nc


_NC = None


def _get_nc():
    global _NC
    if _NC is None:
        _NC = _build_program()
    return _NC


def _qk_slot_perm():
    """Column permutation for the Q|K part of qkv_w: slot 2h <- K head-pair h,
    slot 2h+1 <- Q head-pair h."""
    perm = []
    for hp in range(H // 2):
        perm.extend(range(D + hp * P, D + (hp + 1) * P))      # K slot
        perm.extend(range(hp * P, (hp + 1) * P))              # Q slot
    return np.array(perm)


def _prep_in_maps(x, qkv_w, proj_w, proj_b):
    bf16 = ml_dtypes.bfloat16
    qkv_w = np.asarray(qkv_w)
    perm = _qk_slot_perm()
    wq_perm = np.concatenate([qkv_w[:, perm], qkv_w[:, 2 * D:]], axis=1)
    wq = np.ascontiguousarray(wq_perm.astype(bf16))
    wp = np.ascontiguousarray(np.asarray(proj_w).astype(bf16))
    pb = np.ascontiguousarray(np.asarray(proj_b).astype(np.float32))
    x = np.asarray(x)
    in_maps = []
    for c in range(N_CORES):
        xc = x[c * BPC:(c + 1) * BPC].reshape(T, D).astype(bf16)
        xTc = np.ascontiguousarray(xc.T)  # [D, T] bf16
        in_maps.append({"xT": xTc, "wqkv": wq, "wproj": wp, "bias": pb})
    return in_maps


def _run(x, qkv_w, proj_w, proj_b, **spmd_kwargs):
    nc = _get_nc()
    in_maps = _prep_in_maps(x, qkv_w, proj_w, proj_b)
    res = run_bass_kernel_spmd(nc, in_maps, core_ids=list(range(N_CORES)), **spmd_kwargs)
    # yT is [D, T] per core; transpose back to [T, D] on host.
    y = np.stack(
        [res.results[c]["yT"].astype(np.float32).T for c in range(N_CORES)]
    )
    return y.reshape(B, SEQ, D), res


def kernel(x, qkv_w, proj_w, proj_b):
    y, _ = _run(x, qkv_w, proj_w, proj_b)
    return y
